# revision 1
# baseline (speedup 1.0000x reference)
"""RNN-T joint network (dense MLP) Trainium2 Bass kernel.

Math (per batch row n):
    h = relu(f @ W1t.T + g @ W1p.T + b1t + b1p)    # [N, 512]
    y = h @ W2.T + b2                              # [N, 29]

Strategy: data-parallel over batch N=32768 across 8 NeuronCores (4096
rows/core); weights replicated.  Host-side layout prep: x = concat(f, g)
transposed to [K, N] so contraction K sits on SBUF partitions with zero
on-device transposes; K padded 1344 -> 1408 (11 full 128-tiles).
On-device: h.T[j, n] in PSUM via 11 accumulating matmuls per j-tile
(float32r, 1 cyc/row), bias+relu via ScalarE, second matmul contracts
j into y.T[29, n], bias via ScalarE, DMA out.  Host transposes y back.
"""

import numpy as np

import concourse.bacc as bacc
import concourse.bass as bass  # noqa: F401
import concourse.mybir as mybir
from concourse import tile
from concourse.bass_utils import run_bass_kernel_spmd

TRANS_H, PRED_H, JOINT_H, NUM_LABELS = 1024, 320, 512, 29
BATCH = 32768
N_CORES = 8
N_PER_CORE = BATCH // N_CORES          # 4096
K_TOTAL = TRANS_H + PRED_H             # 1344
K_PAD = 1408                           # 11 * 128
K_TILES = K_PAD // 128                 # 11
J_TILES = JOINT_H // 128               # 4
N_CHUNK = 512                          # fp32 moving-operand / PSUM-bank limit
N_CHUNKS = N_PER_CORE // N_CHUNK       # 8

F32 = mybir.dt.float32
F32R = mybir.dt.float32r

_NC_CACHE = {}


def _build_bass():
    """Build the single-core Bass program (same NEFF runs SPMD on 8 cores)."""
    nc = bacc.Bacc(None)

    xT = nc.dram_tensor("xT", [K_PAD, N_PER_CORE], F32R, kind="ExternalInput")
    w1 = nc.dram_tensor("w1", [K_PAD, JOINT_H], F32R, kind="ExternalInput")
    b1 = nc.dram_tensor("b1", [JOINT_H, 1], F32, kind="ExternalInput")
    w2T = nc.dram_tensor("w2T", [JOINT_H, NUM_LABELS], F32R, kind="ExternalInput")
    b2 = nc.dram_tensor("b2", [NUM_LABELS, 1], F32, kind="ExternalInput")
    yT = nc.dram_tensor("yT", [NUM_LABELS, N_PER_CORE], F32, kind="ExternalOutput")

    # views with the k-tile index explicit: row (k*128 + p) -> [p, k, ...]
    xT3 = xT.rearrange("(k p) n -> p k n", p=128)     # [128, K_TILES, N]
    w13 = w1.rearrange("(k p) j -> p k j", p=128)     # [128, K_TILES, JOINT_H]

    # k-tile split for each x-chunk DMA (2 pieces -> pipeline fill + issue amortized)
    K_SPLITS = [(0, 6), (6, K_TILES)]
    # finer pieces for the pipeline-fill chunk so the first matmuls start early
    K_SPLITS_FILL = [(0, 2), (2, 4), (4, 6), (6, 8), (8, K_TILES)]

    with tile.TileContext(nc) as tc:
        with (
            tc.tile_pool(name="consts", bufs=1) as consts,
            tc.tile_pool(name="xpool", bufs=3) as xpool,
            tc.tile_pool(name="hpool", bufs=2) as hpool,
            tc.tile_pool(name="opool", bufs=2) as opool,
            tc.tile_pool(name="psum_h", bufs=6, space="PSUM") as psum_h,
            tc.tile_pool(name="psum_y", bufs=2, space="PSUM") as psum_y,
        ):
            # ---- replicated constants (ACT-ring DMAs; x rides the SP ring) ----
            w1_sb = consts.tile([128, K_TILES, JOINT_H], F32R, name="w1_sb", tag="w1")
            for (ka, kb) in K_SPLITS_FILL:
                nc.scalar.dma_start(out=w1_sb[:, ka:kb, :], in_=w13[:, ka:kb, :])
            w2_sb = consts.tile([128, J_TILES, NUM_LABELS], F32R, name="w2_sb", tag="w2")
            nc.scalar.dma_start(
                out=w2_sb,
                in_=w2T.rearrange("(j p) l -> p j l", p=128),
            )
            b1_sb = consts.tile([128, J_TILES], F32, name="b1_sb", tag="b1")
            nc.scalar.dma_start(
                out=b1_sb, in_=b1.rearrange("(j p) o -> p (j o)", p=128)
            )
            b2_sb = consts.tile([NUM_LABELS, 1], F32, name="b2_sb", tag="b2")
            nc.scalar.dma_start(out=b2_sb, in_=b2[:, :])

            # ---- main loop over batch chunks of 512 ----
            for c in range(N_CHUNKS):
                n0 = c * N_CHUNK
                x_sb = xpool.tile([128, K_TILES, N_CHUNK], F32R, name="x_sb", tag="x")
                for (ka, kb) in (K_SPLITS_FILL if c <= 2 else K_SPLITS):
                    nc.sync.dma_start(
                        out=x_sb[:, ka:kb, :], in_=xT3[:, ka:kb, n0:n0 + N_CHUNK]
                    )

                h_tiles = []
                for j in range(J_TILES):
                    ph = psum_h.tile([128, N_CHUNK], F32, name=f"ph_{j}", tag="ph")
                    for k in range(K_TILES):
                        nc.tensor.matmul(
                            ph,
                            lhsT=w1_sb[:, k, j * 128:(j + 1) * 128],
                            rhs=x_sb[:, k, :],
                            start=(k == 0),
                            stop=(k == K_TILES - 1),
                        )
                    h_sb = hpool.tile([128, N_CHUNK], F32R, name=f"h_{j}", tag=f"h_{j}")
                    nc.scalar.activation(
                        h_sb, ph, mybir.ActivationFunctionType.Relu,
                        bias=b1_sb[:, j:j + 1],
                    )
                    h_tiles.append(h_sb)

                py = psum_y.tile([NUM_LABELS, N_CHUNK], F32, name="py", tag="py")
                for j in range(J_TILES):
                    nc.tensor.matmul(
                        py,
                        lhsT=w2_sb[:, j, :],
                        rhs=h_tiles[j],
                        start=(j == 0),
                        stop=(j == J_TILES - 1),
                    )
                y_sb = opool.tile([NUM_LABELS, N_CHUNK], F32, name="y_sb", tag="y")
                nc.scalar.activation(
                    y_sb, py, mybir.ActivationFunctionType.Identity, bias=b2_sb
                )
                nc.scalar.dma_start(out=yT[:, n0:n0 + N_CHUNK], in_=y_sb)

    nc.finalize()
    return nc


def _get_nc():
    if "nc" not in _NC_CACHE:
        _NC_CACHE["nc"] = _build_bass()
    return _NC_CACHE["nc"]


def _prep_in_maps(f, g, W1t, b1t, W1p, b1p, W2, b2):
    f2 = np.asarray(f, np.float32).reshape(BATCH, TRANS_H)
    g2 = np.asarray(g, np.float32).reshape(BATCH, PRED_H)

    w1 = np.zeros((K_PAD, JOINT_H), np.float32)
    w1[:TRANS_H] = np.asarray(W1t, np.float32).T
    w1[TRANS_H:K_TOTAL] = np.asarray(W1p, np.float32).T
    b1 = (np.asarray(b1t, np.float32) + np.asarray(b1p, np.float32)).reshape(
        JOINT_H, 1
    )
    w2T = np.ascontiguousarray(np.asarray(W2, np.float32).T)
    b2c = np.asarray(b2, np.float32).reshape(NUM_LABELS, 1)

    in_maps = []
    for core in range(N_CORES):
        sl = slice(core * N_PER_CORE, (core + 1) * N_PER_CORE)
        xT = np.zeros((K_PAD, N_PER_CORE), np.float32)
        xT[:TRANS_H] = f2[sl].T
        xT[TRANS_H:K_TOTAL] = g2[sl].T
        in_maps.append(
            {"xT": xT, "w1": w1, "b1": b1, "w2T": w2T, "b2": b2c}
        )
    return in_maps


def _gather(results):
    y = np.empty((1, BATCH, NUM_LABELS), np.float32)
    for core, r in enumerate(results):
        y[0, core * N_PER_CORE:(core + 1) * N_PER_CORE] = r["yT"].T
    return y


def _run(inputs, trace=False):
    in_maps = _prep_in_maps(
        inputs["f"], inputs["g"], inputs["W1t"], inputs["b1t"],
        inputs["W1p"], inputs["b1p"], inputs["W2"], inputs["b2"],
    )
    res = run_bass_kernel_spmd(
        _get_nc(), in_maps, core_ids=list(range(N_CORES)), trace=trace
    )
    return _gather(res.results), res


def kernel(**inputs) -> np.ndarray:
    out, _ = _run(inputs, trace=False)
    return out



# revision 3
# speedup vs baseline: 1.2758x; 1.2758x over previous
"""RNN-T joint network (dense MLP) Trainium2 Bass kernel.

Math (per batch row n):
    h = relu(f @ W1t.T + g @ W1p.T + b1t + b1p)    # [N, 512]
    y = h @ W2.T + b2                              # [N, 29]

Strategy: data-parallel over batch N=32768 across 8 NeuronCores (4096
rows/core); weights replicated.  Layer 1 runs on the fp8e4 DoubleRow
path (2 k-tiles contracted per matmul instruction) over an *augmented*
contraction that restores near-bf16 accuracy from fp8 operands:

    x  = x_hi + x_lo/16        (both fp8e4; x_lo quantized at 16x scale)
    W1 = (w_hi + w_lo)/128     (both fp8e4; quantized at 128x scale)

    psum = x_hi.(w_hi + w_lo) + (16 x_lo).(8 W1_c)   ~= 128 * x.W1

laid out as 21 x-tiles and 32 weight-tiles of 128 rows, consumed by 16
DoubleRow matmuls per (chunk, j-tile).  Host-side layout prep puts the
contraction on SBUF partitions ([K, N] transposes), so no on-device
transposes.  h is produced by ScalarE (relu + b1 bias + 1/128 scale) in
bf16; layer 2 is a plain bf16 matmul chain, bias b2 via ScalarE, DMA out.
Host transposes y back.  Validated accuracy: rel err ~8e-3 (gate 2e-2).
"""

import numpy as np
import ml_dtypes

import concourse.bacc as bacc
import concourse.bass as bass  # noqa: F401
import concourse.mybir as mybir
from concourse import tile
from concourse.bass_utils import run_bass_kernel_spmd

TRANS_H, PRED_H, JOINT_H, NUM_LABELS = 1024, 320, 512, 29
BATCH = 32768
N_CORES = 8
N_PER_CORE = BATCH // N_CORES          # 4096
K_TOTAL = TRANS_H + PRED_H             # 1344

X_TILES = 21                           # 128-row fp8 x tiles in SBUF
W_TILES = 32                           # 128-row fp8 weight tiles
N_DR = W_TILES // 2                    # 16 DoubleRow matmuls per (chunk, j)
J_TILES = JOINT_H // 128               # 4
N_CHUNK = 512                          # PSUM bank / fp8 moving-operand limit
N_CHUNKS = N_PER_CORE // N_CHUNK       # 8

SW = 128.0                             # W1 quantization scale
SX = 16.0                              # x_lo upscale (W1_c at SW/SX = 8)

# rhs x-tile pair (s, s+1) consumed by DoubleRow matmul d (pairs w-tiles
# (2d, 2d+1)): 5 hi-pass, 5 lo-weight-pass, 5 x_lo-pass, 1 tail.
XS = [0, 2, 4, 6, 8, 0, 2, 4, 6, 8, 11, 13, 15, 17, 19, 10]

F32 = mybir.dt.float32
FP8 = mybir.dt.float8e4
BF16 = mybir.dt.bfloat16
E4NP = ml_dtypes.float8_e4m3
BF16NP = ml_dtypes.bfloat16

_NC_CACHE = {}


def _build_bass():
    """Build the single-core Bass program (same NEFF runs SPMD on 8 cores)."""
    nc = bacc.Bacc(None)

    xT = nc.dram_tensor("xT", [X_TILES * 128, N_PER_CORE], FP8, kind="ExternalInput")
    wa = nc.dram_tensor("wa", [W_TILES * 128, JOINT_H], FP8, kind="ExternalInput")
    b1 = nc.dram_tensor("b1", [JOINT_H, 1], F32, kind="ExternalInput")
    w2T = nc.dram_tensor("w2T", [JOINT_H, NUM_LABELS], BF16, kind="ExternalInput")
    b2 = nc.dram_tensor("b2", [NUM_LABELS, 1], F32, kind="ExternalInput")
    yT = nc.dram_tensor("yT", [NUM_LABELS, N_PER_CORE], F32, kind="ExternalOutput")

    # views with the tile index explicit: row (t*128 + p) -> [p, t, ...]
    xT3 = xT.rearrange("(t p) n -> p t n", p=128)     # [128, X_TILES, N]
    wa3 = wa.rearrange("(t p) j -> p t j", p=128)     # [128, W_TILES, JOINT_H]

    # w-tile splits for the constants DMA (first piece covers the first DRs)
    W_SPLITS = [(0, 4), (4, 10), (10, 20), (20, W_TILES)]
    # x-tile splits per chunk DMA; finer for the pipeline-fill chunks
    X_SPLITS = [(0, 10), (10, X_TILES)]
    X_SPLITS_FILL = [(0, 4), (4, 10), (10, 16), (16, X_TILES)]

    with tile.TileContext(nc) as tc:
        with (
            tc.tile_pool(name="consts", bufs=1) as consts,
            tc.tile_pool(name="xpool", bufs=3) as xpool,
            tc.tile_pool(name="hpool", bufs=2) as hpool,
            tc.tile_pool(name="opool", bufs=2) as opool,
            tc.tile_pool(name="psum_h", bufs=6, space="PSUM") as psum_h,
            tc.tile_pool(name="psum_y", bufs=2, space="PSUM") as psum_y,
        ):
            # ---- replicated constants (ACT-ring DMAs; x rides the SP ring) ----
            wa_sb = consts.tile([128, W_TILES, JOINT_H], FP8, name="wa_sb", tag="wa")
            for (ta, tb) in W_SPLITS:
                nc.scalar.dma_start(out=wa_sb[:, ta:tb, :], in_=wa3[:, ta:tb, :])
            w2_sb = consts.tile([128, J_TILES, NUM_LABELS], BF16, name="w2_sb", tag="w2")
            nc.scalar.dma_start(
                out=w2_sb,
                in_=w2T.rearrange("(j p) l -> p j l", p=128),
            )
            b1_sb = consts.tile([128, J_TILES], F32, name="b1_sb", tag="b1")
            nc.scalar.dma_start(
                out=b1_sb, in_=b1.rearrange("(j p) o -> p (j o)", p=128)
            )
            b2_sb = consts.tile([NUM_LABELS, 1], F32, name="b2_sb", tag="b2")
            nc.scalar.dma_start(out=b2_sb, in_=b2[:, :])

            # ---- main loop over batch chunks of 512 ----
            for c in range(N_CHUNKS):
                n0 = c * N_CHUNK
                x_sb = xpool.tile([128, X_TILES, N_CHUNK], FP8, name="x_sb", tag="x")
                for (ta, tb) in (X_SPLITS_FILL if c <= 1 else X_SPLITS):
                    nc.sync.dma_start(
                        out=x_sb[:, ta:tb, :], in_=xT3[:, ta:tb, n0:n0 + N_CHUNK]
                    )

                h_tiles = []
                for j in range(J_TILES):
                    ph = psum_h.tile([128, N_CHUNK], F32, name=f"ph_{j}", tag="ph")
                    for d in range(N_DR):
                        s = XS[d]
                        nc.tensor.matmul(
                            ph,
                            lhsT=wa_sb[:, 2 * d:2 * d + 2, j * 128:(j + 1) * 128],
                            rhs=x_sb[:, s:s + 2, :],
                            start=(d == 0),
                            stop=(d == N_DR - 1),
                            perf_mode=mybir.MatmulPerfMode.DoubleRow,
                        )
                    h_sb = hpool.tile([128, N_CHUNK], BF16, name=f"h_{j}", tag=f"h_{j}")
                    nc.scalar.activation(
                        h_sb, ph, mybir.ActivationFunctionType.Relu,
                        bias=b1_sb[:, j:j + 1], scale=1.0 / SW,
                    )
                    h_tiles.append(h_sb)

                py = psum_y.tile([NUM_LABELS, N_CHUNK], F32, name="py", tag="py")
                for j in range(J_TILES):
                    nc.tensor.matmul(
                        py,
                        lhsT=w2_sb[:, j, :],
                        rhs=h_tiles[j],
                        start=(j == 0),
                        stop=(j == J_TILES - 1),
                    )
                y_sb = opool.tile([NUM_LABELS, N_CHUNK], F32, name="y_sb", tag="y")
                nc.scalar.activation(
                    y_sb, py, mybir.ActivationFunctionType.Identity, bias=b2_sb
                )
                nc.sync.dma_start(out=yT[:, n0:n0 + N_CHUNK], in_=y_sb)

    nc.finalize()
    return nc


def _get_nc():
    if "nc" not in _NC_CACHE:
        _NC_CACHE["nc"] = _build_bass()
    return _NC_CACHE["nc"]


def _prep_in_maps(f, g, W1t, b1t, W1p, b1p, W2, b2):
    f2 = np.asarray(f, np.float32).reshape(BATCH, TRANS_H)
    g2 = np.asarray(g, np.float32).reshape(BATCH, PRED_H)
    x = np.concatenate([f2, g2], axis=1)                    # [BATCH, K_TOTAL]

    x_hi = x.astype(E4NP)
    x_lo = ((x - x_hi.astype(np.float32)) * SX).astype(E4NP)

    w1 = np.concatenate(
        [np.asarray(W1t, np.float32).T, np.asarray(W1p, np.float32).T], axis=0
    )                                                        # [K_TOTAL, JOINT_H]
    w_hi = (w1 * SW).astype(E4NP)
    w_lo = (w1 * SW - w_hi.astype(np.float32)).astype(E4NP)
    w_c = (w1 * (SW / SX)).astype(E4NP)

    wa = np.zeros((W_TILES * 128, JOINT_H), E4NP)
    wa[0:1280] = w_hi[0:1280]          # A-pass: tiles 0-9
    wa[1280:2560] = w_lo[0:1280]       # B-pass: tiles 10-19
    wa[2560:3584] = w_c[0:1024]        # C-pass (f): tiles 20-27
    wa[3584:3840] = w_c[1024:1280]     # C-pass (g): tiles 28-29
    wa[3840:3904] = w_hi[1280:1344]    # tail tile 30, partitions 0-63
    wa[3904:3968] = w_c[1280:1344]     # tail tile 30, partitions 64-127
    # tile 31 stays zero (second DoubleRow slot of the tail matmul)

    b1c = (np.asarray(b1t, np.float32) + np.asarray(b1p, np.float32)).reshape(
        JOINT_H, 1
    )
    w2T = np.ascontiguousarray(np.asarray(W2, np.float32).T).astype(BF16NP)
    b2c = np.asarray(b2, np.float32).reshape(NUM_LABELS, 1)

    in_maps = []
    for core in range(N_CORES):
        sl = slice(core * N_PER_CORE, (core + 1) * N_PER_CORE)
        xT = np.zeros((X_TILES * 128, N_PER_CORE), E4NP)
        xT[0:1344] = x_hi[sl].T                 # tiles 0-9 + tail tile 10 lo-half
        xT[1344:1408] = x_lo[sl, 1280:1344].T   # tail tile 10, partitions 64-127
        xT[1408:2432] = x_lo[sl, 0:1024].T      # tiles 11-18
        xT[2432:2688] = x_lo[sl, 1024:1280].T   # tiles 19-20
        in_maps.append(
            {"xT": xT, "wa": wa, "b1": b1c, "w2T": w2T, "b2": b2c}
        )
    return in_maps


def _gather(results):
    y = np.empty((1, BATCH, NUM_LABELS), np.float32)
    for core, r in enumerate(results):
        y[0, core * N_PER_CORE:(core + 1) * N_PER_CORE] = r["yT"].T
    return y


def _run(inputs, trace=False):
    in_maps = _prep_in_maps(
        inputs["f"], inputs["g"], inputs["W1t"], inputs["b1t"],
        inputs["W1p"], inputs["b1p"], inputs["W2"], inputs["b2"],
    )
    res = run_bass_kernel_spmd(
        _get_nc(), in_maps, core_ids=list(range(N_CORES)), trace=trace
    )
    return _gather(res.results), res


def kernel(**inputs) -> np.ndarray:
    out, _ = _run(inputs, trace=False)
    return out
